# revision 37
# baseline (speedup 1.0000x reference)
"""MoE-LoRA linear kernel for Trainium2 (8 NeuronCores, data-parallel over tokens).

Computes, for x:[B,S,Din], base_w:[Dout,Din], gate_w:[E,Din],
lora_A:[E*R,Din], lora_B:[Dout,E*R]:

    base   = x @ base_w.T
    logits = x @ gate_w.T ; top-2 renormalized softmax -> dense w:[*,E]
    ax     = x @ lora_A.T                 (per-expert rank-R blocks)
    delta  = (ax * w_expanded) @ lora_B.T * SCALING
    out    = base + delta
Sharding: tokens (B*S=8192) split across 8 cores, 1024 tokens each.
Weights replicated. No collectives.

All matmul operands are fp16 (host-cast); PSUM accumulation is fp32, output
fp32.  fp16 matmuls run at 1 cycle/row on the PE for every shape here (vs 4
for a 32-wide fp32 gating matmul) and halve all DMA traffic.

Structure per core:
  phase 1 (fused, k-major, two 4-tile halves): x and lora_A stream once as
    [128, ...] k-slabs directly into persistent SBUF tiles; per k-slab the PE
    runs 4 ax matmuls (512-wide) + 4 gating matmuls (32-wide) so compute
    hides the stream.  Each half owns a PSUM logits bank + 4 ax banks; the
    second half starts 3 tiles immediately on untouched banks and skews its
    4th ax tile / gating group so their first writes land after the DVE
    frees the first half's banks (no PE head-of-line stall).
  post-half: top-2 renormalized softmax via DVE Max8 + equality masks;
    gate-weight multiply psum->fp16; PE transpose to [rank, tok] layout.
    The second half's DVE chains are emitted before output-stripe 0 of
    phase 2 for tiles t0-t3, so they hide under ~30us of PE matmuls; its
    transposes run after that.
  phase 2 (t-serial per 512-wide output stripe): whole bw stripe loaded as
    one 4MB fp16 DMA (double-buffered); per token tile 32 base + 4 delta
    matmuls accumulate in one PSUM bank, DVE copy out, DMA store.

SCALING is folded into lora_B host-side.
"""
import sys

if "/opt/trn_rl_repo" not in sys.path:
    sys.path.insert(0, "/opt/trn_rl_repo")

import numpy as np

import concourse.bacc as bacc
import concourse.mybir as mybir
import concourse.tile as tile
from concourse import bass_utils
from concourse.bass import ds, ts

B, S, DIN, DOUT = 4, 2048, 4096, 4096
E, R = 32, 16
SCALING = 2.0
NCORES = 8
T = (B * S) // NCORES  # 1024 tokens per core
P = 128
TT = T // P            # 8 token tiles
KT = DIN // P          # 32 contraction tiles
OT = DOUT // 512       # 8 output column tiles
RR = (E * R) // P      # 4 rank tiles
TH = TT // 2           # tiles per half in phase 1
F32 = mybir.dt.float32
F16 = mybir.dt.float16
F8 = mybir.dt.float8e3
RING = 16          # x fp8 conversion ring depth (k-slabs)
LA_SCALE = 64.0    # la is scaled by this on host; 1/LA_SCALE folded into lb

_CACHE = {}


def _build():
    nc = bacc.Bacc("TRN2", target_bir_lowering=False, debug=False)
    xT = nc.dram_tensor("xT", [DIN, T], F16, kind="ExternalInput")
    bwT = nc.dram_tensor("bwT", [DIN, DOUT], F16, kind="ExternalInput")
    gwP = nc.dram_tensor("gwP", [P, KT * E], F16, kind="ExternalInput")
    laT = nc.dram_tensor("laT", [DIN, E * R], F8, kind="ExternalInput")
    lbT = nc.dram_tensor("lbT", [E * R, DOUT], F16, kind="ExternalInput")
    iden = nc.dram_tensor("iden", [P, P], F16, kind="ExternalInput")
    out = nc.dram_tensor("out", [T, DOUT], F32, kind="ExternalOutput")

    xT3 = xT.ap().rearrange("(k p) t -> p k t", p=P)
    gwP3 = gwP.ap().rearrange("p (k e) -> p k e", e=E)
    laT3 = laT.ap().rearrange("(k p) r -> p k r", p=P)
    lbT3 = lbT.ap().rearrange("(rr p) o -> p rr o", p=P)
    bwT3 = bwT.ap().rearrange("(k p) o -> p k o", p=P)
    out2 = out.ap()

    with tile.TileContext(nc, pool_alloc_mode="queue") as tc:
        with (
            tc.tile_pool(name="base", bufs=1) as bp,
            tc.tile_pool(name="psum", bufs=8, space="PSUM") as psum,
        ):
            identity = bp.tile([P, P], F16, tag="iden")
            xsb = bp.tile([P, KT, T], F16, tag="xsb")
            gwt = bp.tile([P, KT, E], F16, tag="gwt")
            axw = bp.tile([P, TT, 512], F16, tag="axw")
            axwT = bp.tile([P, RR, T], F16, tag="axwT")
            wdense = bp.tile([P, TT, E], F32, tag="wd")
            lat = bp.tile([P, KT, E * R], F8, tag="lat")
            xr = bp.tile([P, KT, T // 2], F8, tag="xr")

            # ---- phase 1: stream x/la, fused gating + ax ----
            if True:
                # identity first: it feeds the PE warmup below
                TH2 = T // 2
                # The first phase-1 half only reads tokens 0-511 of each
                # slab (gating t0-3 + the fp8 conversion slice), the second
                # half only tokens 512-1023.  Stream x by token-halves so
                # h0's stream is half the bytes (PE-bound); h1's token-half
                # streams during h0's compute.
                nc.sync.dma_start(identity[:], iden.ap())
                nc.sync.dma_start(xsb[:, 0, ds(0, TH2)], xT3[:, 0, ds(0, TH2)])
                nc.sync.dma_start(gwt[:], gwP3[:])
                nc.sync.dma_start(lat[:, 0, :], laT3[:, 0, :])
                k = 1
                for ck in (1, 2, 4):
                    while k < KT and (ck < 4 or k + ck <= KT):
                        nc.sync.dma_start(
                            xsb[:, ds(k, ck), ds(0, TH2)],
                            xT3[:, ds(k, ck), ds(0, TH2)],
                        )
                        nc.sync.dma_start(
                            lat[:, ds(k, ck), :], laT3[:, ds(k, ck), :]
                        )
                        k += ck
                        if ck < 4:
                            break
                for k in range(0, KT, 8):
                    nc.sync.dma_start(
                        xsb[:, ds(k, 8), ds(TH2, TH2)],
                        xT3[:, ds(k, 8), ds(TH2, TH2)],
                    )

                # warmup: keep the PE busy during the x0/la0 stream and burn
                # the slow-p-state window on throwaway transposes.  Its bank
                # slot is reused by the 4th gating tile (write-after-write).
                warm = psum.tile([P, P], F16, tag="bank", name="warm")
                for _ in range(10):
                    nc.tensor.transpose(warm[:], identity[:], identity[:])

                lps = {}
                axps = {}

                def half_kloop(h):
                    """Per k-slab: 4 ax matmuls + 4 gating matmuls.  Every
                    accumulation group owns a full PSUM bank (matmul start
                    clobbers the whole bank, so groups must never share one).
                    Second half: gating groups inherit the first half's ax
                    banks, so each is skewed until the DVE multiply frees its
                    bank; ax tiles inherit the gating banks (freed by the
                    cheap logits copies at the boundary)."""
                    t0 = h * TH
                    sgs = [3, 5, 7, 9] if h else [0, 0, 0, 0]
                    # Pool converts this half's tokens fp16->fp8; the other
                    # half's conversions overwrite the same buffer later
                    for k in range(KT):
                        eng = nc.gpsimd if k % 2 else nc.vector
                        eng.tensor_copy(
                            xr[:, k, :], xsb[:, k, ds(h * (T // 2), T // 2)]
                        )
                    for i in range(TH):
                        lps[t0 + i] = psum.tile(
                            [P, E], F32, tag="bank", name=f"lps{t0 + i}"
                        )
                    for i in range(TH):
                        axps[t0 + i] = psum.tile(
                            [P, 512], F32, tag="bank", name=f"axps{t0 + i}"
                        )
                    LAG = 1  # ax trails the stream so fp8 conversion latency hides
                    for k in range(KT + LAG):
                        ka = k - LAG
                        if ka >= 0:
                            for i in range(TH):
                                nc.tensor.matmul(
                                    axps[t0 + i][:],
                                    xr[:, ka, ts(i, P)],
                                    lat[:, ka, :],
                                    start=(ka == 0),
                                    stop=(ka == KT - 1),
                                )
                        if k >= KT:
                            continue
                        for i in range(TH):
                            sg = sgs[i]
                            if k < sg:
                                continue
                            nc.tensor.matmul(
                                lps[t0 + i][:],
                                xsb[:, k - sg, ts(t0 + i, P)],
                                gwt[:, k - sg, :],
                                start=(k == sg),
                                stop=(k == KT - 1 and sg == 0),
                            )
                    for i in range(TH):
                        sg = sgs[i]
                        for k in range(KT - sg, KT):
                            nc.tensor.matmul(
                                lps[t0 + i][:],
                                xsb[:, k, ts(t0 + i, P)],
                                gwt[:, k, :],
                                start=False,
                                stop=(k == KT - 1),
                            )

                half_kloop(0)
                half_kloop(1)

            with (
                tc.tile_pool(name="post", bufs=2) as pp,
                tc.tile_pool(name="p2bw", bufs=2) as p2bw,
                tc.tile_pool(name="p2lb", bufs=2) as p2lb,
                tc.tile_pool(name="p2o", bufs=4) as p2o,
            ):
                # PSUM bank choreography: allocation order fixes the (per-tag
                # round-robin) bank each tile inherits; order below pairs
                # every tile with a bank whose previous owner is freed before
                # this tile's first write.
                ps2_pre = {
                    2: psum.tile([P, 512], F32, tag="bank", name="ps2p2"),
                    3: psum.tile([P, 512], F32, tag="bank", name="ps2p3"),
                }
                tq1 = [
                    psum.tile([P, 512], F16, tag="bank", name=f"tq1{j}")
                    for j in range(2)
                ]
                tq0 = [
                    psum.tile([P, 512], F16, tag="bank", name=f"tq0{j}")
                    for j in range(2)
                ]
                ps2_pre[0] = psum.tile([P, 512], F32, tag="bank", name="ps2p0")
                ps2_pre[1] = psum.tile([P, 512], F32, tag="bank", name="ps2p1")

                def chain_lsb(h):
                    """DVE: copy logits out of PSUM (frees the gating banks)."""
                    t0 = h * TH
                    lsb = pp.tile([P, TH * E], F32, tag=f"lsb{h}", bufs=1)
                    for i in range(TH):
                        nc.vector.tensor_copy(lsb[:, ts(i, E)], lps[t0 + i][:])
                    return lsb

                def chain_rest(h, lsb):
                    """DVE: softmax/top2 -> wdense, gate-mult -> axw."""
                    t0 = h * TH
                    for i in range(TH):
                        t = t0 + i
                        ls = lsb[:, ts(i, E)]
                        m8 = pp.tile([P, 8], F32, tag="m8", name="m8")
                        nc.vector.max(out=m8[:], in_=ls)
                        d21 = pp.tile([P, 1], F32, tag="d21", name="d21")
                        nc.vector.tensor_sub(d21[:], m8[:, 1:2], m8[:, 0:1])
                        e2 = pp.tile([P, 1], F32, tag="e2", name="e2")
                        nc.scalar.activation(
                            e2[:], d21[:], mybir.ActivationFunctionType.Exp
                        )
                        den = pp.tile([P, 1], F32, tag="den", name="den")
                        nc.vector.tensor_scalar_add(den[:], e2[:], 1.0)
                        w1 = pp.tile([P, 1], F32, tag="w1", name="w1")
                        nc.vector.reciprocal(w1[:], den[:])
                        w2 = pp.tile([P, 1], F32, tag="w2", name="w2")
                        nc.vector.tensor_mul(w2[:], e2[:], w1[:])
                        eq1 = pp.tile([P, E], F32, tag="eq1", name="eq1")
                        nc.vector.tensor_tensor(
                            eq1[:], ls, m8[:, 0:1].to_broadcast([P, E]),
                            mybir.AluOpType.is_equal,
                        )
                        eq2 = pp.tile([P, E], F32, tag="eq2", name="eq2")
                        nc.vector.tensor_tensor(
                            eq2[:], ls, m8[:, 1:2].to_broadcast([P, E]),
                            mybir.AluOpType.is_equal,
                        )
                        nc.vector.tensor_tensor(
                            eq1[:], eq1[:], w1[:].to_broadcast([P, E]),
                            mybir.AluOpType.mult,
                        )
                        nc.vector.tensor_tensor(
                            eq2[:], eq2[:], w2[:].to_broadcast([P, E]),
                            mybir.AluOpType.mult,
                        )
                        nc.vector.tensor_add(wdense[:, t, :], eq1[:], eq2[:])
                        nc.vector.tensor_tensor(
                            axw[:, t, :].rearrange("p (e r) -> p e r", r=R),
                            axps[t][:].rearrange("p (e r) -> p e r", r=R),
                            wdense[:, t, :, None].to_broadcast([P, E, R]),
                            mybir.AluOpType.mult,
                        )

                def transposes(h, tq):
                    """PE transposes axw -> axwT via 2 ping-pong PSUM banks."""
                    t0 = h * TH
                    for i in range(TH):
                        t = t0 + i
                        tpq = tq[i % 2]
                        for rr in range(RR):
                            nc.tensor.transpose(
                                tpq[:, ts(rr, P)], axw[:, t, ts(rr, P)],
                                identity[:],
                            )
                        nc.vector.tensor_copy(
                            axwT[:, :, ts(t, P)],
                            tpq[:].rearrange("p (rr q) -> p rr q", q=P),
                        )

                def load_bw(o):
                    bws = p2bw.tile([P, KT, 512], F16, tag="bw", name="bw")
                    nc.sync.dma_start(bws[:], bwT3[:, :, ds(o * 512, 512)])
                    return bws

                def load_lb(o):
                    lb = p2lb.tile([P, RR, 512], F16, tag="lb", name="lb")
                    nc.sync.dma_start(lb[:], lbT3[:, :, ds(o * 512, 512)])
                    return lb

                def out_tile(o, t, bw_cur, lb_cur, width=512, coff=0, ps2=None):
                    if ps2 is None:
                        ps2 = psum.tile([P, width], F32, tag="bank", name="ps2")
                    else:
                        ps2 = ps2[:, :width]
                    for k in range(KT):
                        nc.tensor.matmul(
                            ps2[:],
                            xsb[:, k, ts(t, P)],
                            bw_cur[:, k, ds(coff, width)],
                            start=(k == 0), stop=False,
                        )
                    for rr in range(RR):
                        nc.tensor.matmul(
                            ps2[:],
                            axwT[:, rr, ts(t, P)],
                            lb_cur[:, rr, ds(coff, width)],
                            start=False, stop=(rr == RR - 1),
                        )
                    osb = p2o.tile([P, width], F32, tag="osb", name="osb")
                    nc.vector.tensor_copy(osb[:], ps2[:])
                    nc.sync.dma_start(
                        out2[ts(t, P), ds(o * 512 + coff, width)], osb[:]
                    )

                bw_cur = load_bw(0)
                lb_cur = load_lb(0)
                bw_nxt = load_bw(1)
                lb_nxt = load_lb(1)

                lsb0 = chain_lsb(0)
                chain_rest(0, lsb0)
                lsb1 = chain_lsb(1)
                transposes(0, tq0)
                chain_rest(1, lsb1)
                for t in range(TH):
                    out_tile(0, t, bw_cur, lb_cur, ps2=ps2_pre[t])
                transposes(1, tq1)
                for t in range(TH, TT):
                    out_tile(0, t, bw_cur, lb_cur)
                bw_cur, lb_cur = bw_nxt, lb_nxt

                for o in range(1, OT):
                    if o + 1 < OT:
                        bw_nxt = load_bw(o + 1)
                        lb_nxt = load_lb(o + 1)
                    for t in range(TT):
                        if o == OT - 1 and t == TT - 1:
                            # final tile in four column chunks so each chunk's
                            # copy+store overlaps the next chunk's matmuls
                            for j in range(4):
                                out_tile(
                                    o, t, bw_cur, lb_cur,
                                    width=128, coff=128 * j,
                                )
                        else:
                            out_tile(o, t, bw_cur, lb_cur)
                    bw_cur, lb_cur = bw_nxt, lb_nxt

    nc.compile()
    return nc


def _get_nc():
    if "nc" not in _CACHE:
        _CACHE["nc"] = _build()
    return _CACHE["nc"]


def kernel(x, base_w, gate_w, lora_A, lora_B):
    nc = _get_nc()

    x2 = np.asarray(x, dtype=np.float32).reshape(B * S, DIN)
    bwT = np.ascontiguousarray(
        np.asarray(base_w, dtype=np.float32).T.astype(np.float16)
    )
    # gate_w packed so each SBUF partition's [KT, E] block is contiguous:
    # gwP[p, k*E + e] = gate_w[e, k*128 + p]
    gwP = np.ascontiguousarray(
        np.asarray(gate_w, dtype=np.float32)
        .T.reshape(KT, P, E)
        .transpose(1, 0, 2)
        .reshape(P, KT * E)
        .astype(np.float16)
    )
    import ml_dtypes

    laT = np.ascontiguousarray(
        (np.asarray(lora_A, dtype=np.float32).T * np.float32(LA_SCALE)).astype(
            ml_dtypes.float8_e3m4
        )
    )
    lbT = np.ascontiguousarray(
        (
            np.asarray(lora_B, dtype=np.float32).T
            * np.float32(SCALING / LA_SCALE)
        ).astype(np.float16)
    )
    iden = np.eye(P, dtype=np.float16)

    in_maps = []
    for c in range(NCORES):
        xT_c = np.ascontiguousarray(
            x2[c * T : (c + 1) * T].T.astype(np.float16)
        )
        in_maps.append(
            {
                "xT": xT_c,
                "bwT": bwT,
                "gwP": gwP,
                "laT": laT,
                "lbT": lbT,
                "iden": iden,
            }
        )

    res = bass_utils.run_bass_kernel_spmd(nc, in_maps, core_ids=list(range(NCORES)))
    parts = [res.results[c]["out"] for c in range(NCORES)]
    return np.concatenate(parts, axis=0).reshape(B, S, DOUT).astype(np.float32)
